# revision 23
# baseline (speedup 1.0000x reference)
"""GraphTransformer (2x PyG TransformerConv + out proj) on 8 trn2 NeuronCores.

Strategy (edge-parallel via dst-ownership):
- Host: sort nodes globally by (degree, id); rank r -> core r%8, local slot
  r//8. Every core's tile t covers the same global rank block => identical
  per-tile max degree D_t on all cores (SPMD-compatible). Edges grouped by
  dst; each dst's edges live entirely on its owner core as gather slots.
- Device: bf16 throughout (fp32 accumulate), fused k|v table AllGathered in
  two row-chunks so the collective overlaps the producing compute. Per
  128-node tile: indirect-DMA gather of k|v rows per pow2 slot chunk, DVE
  dot via mult + pairwise fold + reduce, exp on scalar engine (softmax is
  shift invariant; logits are small, so no segment max needed). Invalid
  slots gather a zeroed padding row => contribute exp(0)=1 to the denom,
  corrected by the host-known invalid count. Weighted sums fold pairwise in
  bf16, accumulate fp32, then skip-add + ReLU. The PE-transpose blocks of h
  double as the stationary operand for the NEXT layer's projections, fused
  into the same tile loop (no hT round trip); layer-2's edge loop fuses the
  final output projection the same way.
"""
import numpy as np
import ml_dtypes

N, E, D, H, C, HC = 50000, 400000, 384, 4, 128, 512
NCORES, P = 8, 128
NLOC = N // NCORES
NTILES = (NLOC + P - 1) // P
SHARD = NTILES * P
NPAD = SHARD * NCORES
SCHUNK = 8
INV_SQRT_C = 1.0 / np.sqrt(np.float32(C))
BF16 = ml_dtypes.bfloat16
PAD_ROW = SHARD - 1   # core 0's (zeroed) padding row


def _pow2_chunks(d):
    out, s = [], SCHUNK
    while d > 0:
        while s > d:
            s //= 2
        out.append(s)
        d -= s
    return out


# ---------------------------------------------------------------- host prep
def _prep(edge_index):
    src = np.asarray(edge_index[0], dtype=np.int64)
    dst = np.asarray(edge_index[1], dtype=np.int64)
    deg = np.bincount(dst, minlength=N)
    node_of_rank = np.lexsort((np.arange(N), deg))
    rank_of_node = np.empty(N, np.int64)
    rank_of_node[node_of_rank] = np.arange(N)
    g_of_rank = (np.arange(N) % NCORES) * SHARD + (np.arange(N) // NCORES)
    g_of_node = np.empty(N, np.int64)
    g_of_node[node_of_rank] = g_of_rank

    deg_sorted = deg[node_of_rank]
    Dts = []
    for t in range(NTILES):
        blk = deg_sorted[t * P * NCORES:(t + 1) * P * NCORES]
        Dts.append(max(int(blk.max()) if len(blk) else 0, 1))
    SUMD = sum(Dts)
    coloff = np.cumsum([0] + Dts)[:-1]

    er = rank_of_node[dst]
    order = np.argsort(er, kind="stable")
    er_s = er[order]
    gsrc_s = g_of_node[src[order]]
    starts = np.searchsorted(er_s, np.arange(N))
    slot = np.arange(E) - starts[er_s]

    core_e = er_s % NCORES
    local_e = er_s // NCORES
    col_e = coloff[local_e // P] + slot
    p_e = local_e % P

    # invalid slots gather the (zeroed) padding row
    srcidx = np.full((NCORES, P, SUMD), PAD_ROW, np.int32)
    valid = np.zeros((NCORES, P, SUMD), np.float32)
    srcidx[core_e, p_e, col_e] = gsrc_s.astype(np.int32)
    valid[core_e, p_e, col_e] = 1.0
    # per-(core, p, tile): -#invalid slots, added to the denominator
    ninv = np.zeros((NCORES, P, NTILES), np.float32)
    for t in range(NTILES):
        a, b = int(coloff[t]), int(coloff[t]) + Dts[t]
        ninv[:, :, t] = -(Dts[t] - valid[:, :, a:b].sum(axis=2))
    return srcidx, ninv, Dts, SUMD, coloff, node_of_rank


def _shard_rows(x, node_of_rank):
    D_in = x.shape[1]
    out = np.zeros((NCORES, SHARD, D_in), np.float32)
    r = np.arange(N)
    out[r % NCORES, r // NCORES] = x[node_of_rank]
    return out


# ---------------------------------------------------------------- wait fix
def _split_waits(nc):
    """walrus here rejects >1 sem-wait per instruction; split extras onto
    InstNoOp carriers inserted just before, same engine."""
    import concourse.mybir as mybir
    for fn in nc.m.functions:
        for bb in fn.blocks:
            out = []
            changed = False
            for ins in bb.instructions:
                si = ins.sync_info
                waits = list(si.on_wait) if si and si.on_wait else []
                if len(waits) > 1:
                    changed = True
                    for j, w in enumerate(waits[:-1]):
                        out.append(mybir.InstNoOp(
                            name=f"{ins.name}-wf{j}", opcode="NoOp",
                            engine=ins.engine,
                            sync_info=mybir.SyncInfo(on_wait=[w], on_update=[]),
                            text_hint="waitfix"))
                    si.on_wait = waits[-1:]
                out.append(ins)
            if changed:
                bb.instructions = out


# ---------------------------------------------------------------- bass build
def _build_nc(Dts, SUMD, coloff):
    import concourse.bass as bass
    import concourse.mybir as mybir
    import concourse.tile as tile
    from concourse.masks import make_identity
    f32 = mybir.dt.float32
    bf16 = mybir.dt.bfloat16

    nc = bass.Bass(num_devices=NCORES)
    xT = nc.dram_tensor("xT", [D, SHARD], bf16, kind="ExternalInput")
    srcidx_d = nc.dram_tensor("srcidx", [P, SUMD], mybir.dt.int32, kind="ExternalInput")
    ninv_d = nc.dram_tensor("ninv", [P, NTILES], f32, kind="ExternalInput")
    wT, bia = {}, {}
    for l, Din in ((0, D), (1, HC)):
        for nm in "qkvs":
            wT[nm, l] = nc.dram_tensor(f"w{nm}{l}T", [Din, HC], bf16, kind="ExternalInput")
            bia[nm, l] = nc.dram_tensor(f"b{nm}{l}", [1, HC], bf16, kind="ExternalInput")
    woutT = nc.dram_tensor("woutT", [HC, D], bf16, kind="ExternalInput")
    bout = nc.dram_tensor("bout", [1, D], bf16, kind="ExternalInput")
    out_d = nc.dram_tensor("out", [SHARD, D], f32, kind="ExternalOutput")

    DTMAX = max(Dts)
    chunks = []  # per tile: list of (abs_col, local_off, S)
    for t in range(NTILES):
        cs, off = [], 0
        for s in _pow2_chunks(Dts[t]):
            cs.append((int(coloff[t]) + off, off, s))
            off += s
        chunks.append(cs)

    rg = [list(range(NCORES))]

    with tile.TileContext(nc) as tc:
        with (
            tc.tile_pool(name="dram", bufs=1, space="DRAM") as dram,
            tc.tile_pool(name="const", bufs=1) as const,
        ):
            # persistent DRAM scratch
            qsd = [dram.tile([SHARD, 2 * HC], bf16, name=f"qs{l}d") for l in range(2)]
            kvin = [dram.tile([SHARD, 2 * HC], bf16, name=f"kv{l}in") for l in range(2)]
            kvfull = [dram.tile([NPAD, 2 * HC], bf16, name=f"kv{l}full", addr_space="Shared")
                      for l in range(2)]
            hT0 = dram.tile([HC, SHARD], bf16, name="h0T")

            # constants + all weights in SBUF up front
            ident = const.tile([P, P], bf16)
            make_identity(nc, ident[:])
            ones = const.tile([1, P], bf16)
            nc.vector.memset(ones[:], 1.0)
            srcidx_s = const.tile([P, SUMD], mybir.dt.int32)
            nc.sync.dma_start(srcidx_s[:], srcidx_d[:])
            ninv_s = const.tile([P, NTILES], f32)
            nc.sync.dma_start(ninv_s[:], ninv_d[:])
            w_s, bias_s = {}, {}
            for l, Din in ((0, D), (1, HC)):
                KB = Din // P
                for nm in "qkvs":
                    w_s[nm, l] = const.tile([P, KB * HC], bf16, name=f"w{nm}{l}s")
                    nc.sync.dma_start(
                        w_s[nm, l][:].rearrange("p (kb n) -> p kb n", n=HC),
                        wT[nm, l][:].rearrange("(kb p) n -> p kb n", p=P))
                    bias_s[nm, l] = const.tile([1, HC], bf16, name=f"b{nm}{l}s")
                    nc.sync.dma_start(bias_s[nm, l][:], bia[nm, l][:])
            wo_s = const.tile([P, (HC // P) * D], bf16)
            nc.sync.dma_start(
                wo_s[:].rearrange("p (kb n) -> p kb n", n=D),
                woutT[:].rearrange("(kb p) n -> p kb n", p=P))
            bout_s = const.tile([1, D], bf16)
            nc.sync.dma_start(bout_s[:], bout[:])

            def ag(l):
                nc.gpsimd.collective_compute(
                    "AllGather", mybir.AluOpType.bypass, replica_groups=rg,
                    ins=[kvin[l][:].opt()], outs=[kvfull[l][:].opt()])

            def proj_tile(l, t, group, lhsT, lp, op, pp):
                """Project one 128-row tile for group ('kv'|'qs') from a
                [P, KB*P] stationary operand already in SBUF (or load it)."""
                KB = (D if l == 0 else HC) // P
                rows = slice(t * P, (t + 1) * P)
                o = op.tile([P, 2 * HC], bf16, tag=f"o{group}")
                for j, nm in enumerate(group):
                    ps = pp.tile([P, HC], f32, tag="ps")
                    for kb in range(KB):
                        nc.tensor.matmul(
                            ps[:], lhsT=lhsT[:, kb * P:(kb + 1) * P],
                            rhs=w_s[nm, l][:].rearrange("p (kb n) -> p kb n", n=HC)[:, kb, :],
                            start=(kb == 0), stop=False)
                    nc.tensor.matmul(
                        ps[:], lhsT=ones[:1, :], rhs=bias_s[nm, l][:1, :],
                        start=False, stop=True)
                    nc.scalar.copy(o[:, j * HC:(j + 1) * HC], ps[:])
                if group == "kv":
                    nc.sync.dma_start(kvin[l][rows, :], o[:])
                else:
                    nc.sync.dma_start(qsd[l][rows, :], o[:])

            def load_lhs0(t, lp):
                KB = D // P
                lt = lp.tile([P, KB * P], bf16, tag="lhs")
                nc.sync.dma_start(
                    lt[:].rearrange("p (kb r) -> p kb r", r=P),
                    xT[:, t * P:(t + 1) * P].rearrange("(kb p) r -> p kb r", p=P))
                return lt

            def zpad_write(l, op):
                zp = op.tile([1, 2 * HC], bf16, tag="zpad")
                nc.vector.memset(zp[:], 0.0)
                nc.sync.dma_start(kvin[l][SHARD - 1:SHARD, :], zp[:])

            # ---------------- layer-0 projections: kv -> AG0 -> qs --------
            with (
                tc.tile_pool(name="lp0", bufs=3) as lp0,
                tc.tile_pool(name="op0", bufs=2) as op0,
                tc.tile_pool(name="pp0", bufs=2, space="PSUM") as pp0,
            ):
                for t in range(NTILES):
                    proj_tile(0, t, "kv", load_lhs0(t, lp0)[:], lp0, op0, pp0)
                zpad_write(0, op0)
                ag(0)
                for t in range(NTILES):
                    proj_tile(0, t, "qs", load_lhs0(t, lp0)[:], lp0, op0, pp0)

            # ---------------- edge phases ---------------------------------
            def edge_phase(l):
                with (
                    tc.tile_pool(name=f"ek{l}", bufs=3) as ek,
                    tc.tile_pool(name=f"eg{l}", bufs=2) as eg,
                    tc.tile_pool(name=f"eh{l}", bufs=2) as eh,
                    tc.tile_pool(name=f"eo{l}", bufs=2) as eo,
                    tc.tile_pool(name=f"et{l}", bufs=4, space="PSUM") as et,
                    tc.tile_pool(name=f"ep{l}", bufs=2, space="PSUM") as ep,
                ):
                    for t in range(NTILES):
                        rows = slice(t * P, (t + 1) * P)
                        qs_t = eg.tile([P, 2 * HC], bf16, tag="qs")
                        nc.sync.dma_start(qs_t[:], qsd[l][rows, :])
                        ebuf = eh.tile([P, DTMAX * H], bf16, tag="ebuf")
                        msg = eh.tile([P, HC], f32, tag="msg")
                        for ci, (co, lo, S) in enumerate(chunks[t]):
                            kvg = ek.tile([P, SCHUNK * 2 * HC], bf16, tag="kvg")
                            for s in range(S):
                                nc.gpsimd.indirect_dma_start(
                                    out=kvg[:, s * 2 * HC:(s + 1) * 2 * HC],
                                    out_offset=None,
                                    in_=kvfull[l][:],
                                    in_offset=bass.IndirectOffsetOnAxis(
                                        ap=srcidx_s[:, co + s:co + s + 1], axis=0))
                            kv3 = kvg[:].rearrange("p (s kv) -> p s kv", kv=2 * HC)
                            prod = eg.tile([P, SCHUNK * HC], bf16, tag="prod")
                            nc.vector.tensor_tensor(
                                out=prod[:].rearrange("p (s n) -> p s n", n=HC)[:, :S],
                                in0=kv3[:, :S, 0:HC],
                                in1=qs_t[:, None, 0:HC].to_broadcast([P, S, HC]),
                                op=mybir.AluOpType.mult)
                            p5 = prod[:].rearrange(
                                "p (s h two c) -> p s h two c", h=H, two=2, c=C // 2)
                            foldc = eg.tile([P, SCHUNK * H * (C // 2)], bf16, tag="foldc")
                            nc.vector.tensor_tensor(
                                out=foldc[:].rearrange(
                                    "p (s h c) -> p s h c", h=H, c=C // 2)[:, :S],
                                in0=p5[:, :S, :, 0, :], in1=p5[:, :S, :, 1, :],
                                op=mybir.AluOpType.add)
                            alpha = eh.tile([P, SCHUNK * H], f32, tag="alpha")
                            nc.vector.tensor_reduce(
                                alpha[:, :S * H],
                                foldc[:, :S * H * (C // 2)].rearrange(
                                    "p (sh c) -> p sh c", c=C // 2),
                                axis=mybir.AxisListType.X, op=mybir.AluOpType.add)
                            nc.scalar.activation(
                                ebuf[:, lo * H:(lo + S) * H], alpha[:, :S * H],
                                mybir.ActivationFunctionType.Exp, scale=float(INV_SQRT_C))
                            ev = eg.tile([P, SCHUNK * HC], bf16, tag="prod")
                            nc.vector.tensor_tensor(
                                out=ev[:].rearrange("p (s h c) -> p s h c", h=H, c=C)[:, :S],
                                in0=kv3[:, :S, HC:2 * HC].rearrange("p s (h c) -> p s h c", c=C),
                                in1=ebuf[:, lo * H:(lo + S) * H]
                                    .rearrange("p (s h) -> p s h", h=H)
                                    [:, :, :, None].to_broadcast([P, S, H, C]),
                                op=mybir.AluOpType.mult)
                            n = S
                            while n > 1:
                                half = n // 2
                                nc.vector.tensor_tensor(
                                    out=ev[:, :half * HC].rearrange("p (s n) -> p s n", n=HC),
                                    in0=ev[:, :half * HC].rearrange("p (s n) -> p s n", n=HC),
                                    in1=ev[:, half * HC:n * HC].rearrange("p (s n) -> p s n", n=HC),
                                    op=mybir.AluOpType.add)
                                n = half
                            if ci == 0:
                                nc.scalar.copy(msg[:], ev[:, :HC])
                            else:
                                nc.vector.tensor_add(msg[:], msg[:], ev[:, :HC])
                        den = eh.tile([P, H], f32, tag="den")
                        nc.vector.tensor_reduce(
                            den[:], ebuf[:, :Dts[t] * H].rearrange("p (s h) -> p h s", h=H),
                            axis=mybir.AxisListType.X, op=mybir.AluOpType.add)
                        # add -#invalid (each contributed exp(0)=1); clamp off 0
                        nc.vector.tensor_add(
                            den[:], den[:], ninv_s[:, t:t + 1].to_broadcast([P, H]))
                        nc.vector.tensor_scalar_max(den[:], den[:], 1e-16)
                        rden = eh.tile([P, H], f32, tag="rden")
                        nc.vector.reciprocal(rden[:], den[:])
                        h_t = eh.tile([P, HC], bf16, tag="h")
                        nc.vector.tensor_tensor(
                            out=h_t[:].rearrange("p (h c) -> p h c", c=C),
                            in0=msg[:].rearrange("p (h c) -> p h c", c=C),
                            in1=rden[:, :, None].to_broadcast([P, H, C]),
                            op=mybir.AluOpType.mult)
                        nc.vector.tensor_add(h_t[:], h_t[:], qs_t[:, HC:2 * HC])
                        nc.scalar.activation(h_t[:], h_t[:], mybir.ActivationFunctionType.Relu)
                        o4 = eh.tile([P, (HC // P) * P], bf16, tag="tpo")
                        for kb in range(HC // P):
                            tp = et.tile([P, P], bf16, tag="tp")
                            nc.tensor.transpose(tp[:], h_t[:, kb * P:(kb + 1) * P], ident[:])
                            nc.scalar.copy(o4[:, kb * P:(kb + 1) * P], tp[:])
                        if l == 0:
                            # layer-1 k|v projection straight from the
                            # transpose; store hT0 for the later q|s pass
                            proj_tile(1, t, "kv", o4[:], None, eo, ep)
                            nc.sync.dma_start(
                                hT0[:].rearrange("(kb p) r -> p kb r", p=P)
                                [:, :, t * P:(t + 1) * P],
                                o4[:].rearrange("p (kb r) -> p kb r", r=P))
                        else:
                            # final projection: out = h @ woutT + bout
                            ps = ep.tile([P, D], f32, tag="po")
                            for kb in range(HC // P):
                                nc.tensor.matmul(
                                    ps[:], lhsT=o4[:, kb * P:(kb + 1) * P],
                                    rhs=wo_s[:].rearrange("p (kb n) -> p kb n", n=D)[:, kb, :],
                                    start=(kb == 0), stop=False)
                            nc.tensor.matmul(ps[:], lhsT=ones[:1, :], rhs=bout_s[:1, :],
                                             start=False, stop=True)
                            oo = eo.tile([P, D], f32, tag="oo")
                            nc.scalar.copy(oo[:], ps[:])
                            nc.sync.dma_start(out_d[rows, :], oo[:])
                    if l == 0:
                        zpad_write(1, eo)

            edge_phase(0)
            ag(1)
            # q|s-1 projections from stored hT0, overlapping AG1
            with (
                tc.tile_pool(name="lp1", bufs=3) as lp1,
                tc.tile_pool(name="op1", bufs=2) as op1,
                tc.tile_pool(name="pp1", bufs=2, space="PSUM") as pp1,
            ):
                KB = HC // P
                for t in range(NTILES):
                    lt = lp1.tile([P, KB * P], bf16, tag="lhs")
                    nc.sync.dma_start(
                        lt[:].rearrange("p (kb r) -> p kb r", r=P),
                        hT0[:, t * P:(t + 1) * P].rearrange("(kb p) r -> p kb r", p=P))
                    proj_tile(1, t, "qs", lt[:], lp1, op1, pp1)
            edge_phase(1)

    _split_waits(nc)
    return nc


def _make_in_maps(inputs, srcidx, ninv, node_of_rank):
    x = np.ascontiguousarray(np.asarray(inputs["x"], np.float32))
    xsh = _shard_rows(x, node_of_rank)
    g = lambda n: np.ascontiguousarray(np.asarray(inputs[n], np.float32))
    common = {}
    for l in range(2):
        for nm in "qkvs":
            common[f"w{nm}{l}T"] = np.ascontiguousarray(g(f"{nm}{l}_w").T).astype(BF16)
            common[f"b{nm}{l}"] = g(f"{nm}{l}_b").reshape(1, HC).astype(BF16)
    common["woutT"] = np.ascontiguousarray(g("out_w").T).astype(BF16)
    common["bout"] = g("out_b").reshape(1, D).astype(BF16)
    in_maps = []
    for c in range(NCORES):
        m = dict(common)
        m["xT"] = np.ascontiguousarray(xsh[c].T).astype(BF16)
        m["srcidx"] = np.ascontiguousarray(srcidx[c])
        m["ninv"] = np.ascontiguousarray(ninv[c])
        in_maps.append(m)
    return in_maps


def kernel(**inputs):
    from concourse.bass_utils import run_bass_kernel_spmd
    srcidx, ninv, Dts, SUMD, coloff, node_of_rank = _prep(np.asarray(inputs["edge_index"]))
    nc = _build_nc(Dts, SUMD, coloff)
    in_maps = _make_in_maps(inputs, srcidx, ninv, node_of_rank)
    res = run_bass_kernel_spmd(nc, in_maps, core_ids=list(range(NCORES)))
    shards = np.stack([res.results[c]["out"] for c in range(NCORES)])
    full = np.empty((N, D), np.float32)
    r = np.arange(N)
    full[node_of_rank] = shards[r % NCORES, r // NCORES]
    return full


# revision 24
# speedup vs baseline: 1.2176x; 1.2176x over previous
"""GraphTransformer (2x PyG TransformerConv + out proj) on 8 trn2 NeuronCores.

Strategy (edge-parallel via dst-ownership):
- Host: sort nodes globally by (degree, id); rank r -> core r%8, local slot
  r//8. Every core's tile t covers the same global rank block => identical
  per-tile max degree D_t on all cores (SPMD-compatible). Edges grouped by
  dst; each dst's edges live entirely on its owner core as gather slots.
- Device: bf16 throughout (fp32 accumulate), fused k|v table AllGathered in
  two row-chunks so the collective overlaps the producing compute. Per
  128-node tile: indirect-DMA gather of k|v rows per pow2 slot chunk, DVE
  dot via mult + pairwise fold + reduce, exp on scalar engine (softmax is
  shift invariant; logits are small, so no segment max needed). Invalid
  slots gather a zeroed padding row => contribute exp(0)=1 to the denom,
  corrected by the host-known invalid count. Weighted sums fold pairwise in
  bf16, accumulate fp32, then skip-add + ReLU. The PE-transpose blocks of h
  double as the stationary operand for the NEXT layer's projections, fused
  into the same tile loop (no hT round trip); layer-2's edge loop fuses the
  final output projection the same way.
"""
import numpy as np
import ml_dtypes

N, E, D, H, C, HC = 50000, 400000, 384, 4, 128, 512
NCORES, P = 8, 128
NLOC = N // NCORES
NTILES = (NLOC + P - 1) // P
SHARD = NTILES * P
NPAD = SHARD * NCORES
SCHUNK = 8
INV_SQRT_C = 1.0 / np.sqrt(np.float32(C))
BF16 = ml_dtypes.bfloat16
PAD_ROW = SHARD - 1   # core 0's (zeroed) padding row


def _pow2_chunks(d):
    out, s = [], SCHUNK
    while d > 0:
        while s > d:
            s //= 2
        out.append(s)
        d -= s
    return out


# ---------------------------------------------------------------- host prep
def _prep(edge_index):
    src = np.asarray(edge_index[0], dtype=np.int64)
    dst = np.asarray(edge_index[1], dtype=np.int64)
    deg = np.bincount(dst, minlength=N)
    node_of_rank = np.lexsort((np.arange(N), deg))
    rank_of_node = np.empty(N, np.int64)
    rank_of_node[node_of_rank] = np.arange(N)
    g_of_rank = (np.arange(N) % NCORES) * SHARD + (np.arange(N) // NCORES)
    g_of_node = np.empty(N, np.int64)
    g_of_node[node_of_rank] = g_of_rank

    deg_sorted = deg[node_of_rank]
    Dts = []
    for t in range(NTILES):
        blk = deg_sorted[t * P * NCORES:(t + 1) * P * NCORES]
        Dts.append(max(int(blk.max()) if len(blk) else 0, 1))
    SUMD = sum(Dts)
    coloff = np.cumsum([0] + Dts)[:-1]

    er = rank_of_node[dst]
    order = np.argsort(er, kind="stable")
    er_s = er[order]
    gsrc_s = g_of_node[src[order]]
    starts = np.searchsorted(er_s, np.arange(N))
    slot = np.arange(E) - starts[er_s]

    core_e = er_s % NCORES
    local_e = er_s // NCORES
    col_e = coloff[local_e // P] + slot
    p_e = local_e % P

    # invalid slots gather the (zeroed) padding row
    srcidx = np.full((NCORES, P, SUMD), PAD_ROW, np.int32)
    valid = np.zeros((NCORES, P, SUMD), np.float32)
    srcidx[core_e, p_e, col_e] = gsrc_s.astype(np.int32)
    valid[core_e, p_e, col_e] = 1.0
    # per-(core, p, tile): -#invalid slots, added to the denominator
    ninv = np.zeros((NCORES, P, NTILES), np.float32)
    for t in range(NTILES):
        a, b = int(coloff[t]), int(coloff[t]) + Dts[t]
        ninv[:, :, t] = -(Dts[t] - valid[:, :, a:b].sum(axis=2))
    return srcidx, ninv, Dts, SUMD, coloff, node_of_rank


def _shard_rows(x, node_of_rank):
    D_in = x.shape[1]
    out = np.zeros((NCORES, SHARD, D_in), np.float32)
    r = np.arange(N)
    out[r % NCORES, r // NCORES] = x[node_of_rank]
    return out


# ---------------------------------------------------------------- wait fix
def _split_waits(nc):
    """walrus here rejects >1 sem-wait per instruction; split extras onto
    InstNoOp carriers inserted just before, same engine."""
    import concourse.mybir as mybir
    for fn in nc.m.functions:
        for bb in fn.blocks:
            out = []
            changed = False
            for ins in bb.instructions:
                si = ins.sync_info
                waits = list(si.on_wait) if si and si.on_wait else []
                if len(waits) > 1:
                    changed = True
                    for j, w in enumerate(waits[:-1]):
                        out.append(mybir.InstNoOp(
                            name=f"{ins.name}-wf{j}", opcode="NoOp",
                            engine=ins.engine,
                            sync_info=mybir.SyncInfo(on_wait=[w], on_update=[]),
                            text_hint="waitfix"))
                    si.on_wait = waits[-1:]
                out.append(ins)
            if changed:
                bb.instructions = out


# ---------------------------------------------------------------- bass build
def _build_nc(Dts, SUMD, coloff):
    import concourse.bass as bass
    import concourse.mybir as mybir
    import concourse.tile as tile
    from concourse.masks import make_identity
    f32 = mybir.dt.float32
    bf16 = mybir.dt.bfloat16

    nc = bass.Bass(num_devices=NCORES)
    xT = nc.dram_tensor("xT", [SHARD, D], bf16, kind="ExternalInput")
    srcidx_d = nc.dram_tensor("srcidx", [P, SUMD], mybir.dt.int32, kind="ExternalInput")
    ninv_d = nc.dram_tensor("ninv", [P, NTILES], f32, kind="ExternalInput")
    wT, bia = {}, {}
    for l, Din in ((0, D), (1, HC)):
        for nm in "qkvs":
            wT[nm, l] = nc.dram_tensor(f"w{nm}{l}T", [Din, HC], bf16, kind="ExternalInput")
            bia[nm, l] = nc.dram_tensor(f"b{nm}{l}", [1, HC], bf16, kind="ExternalInput")
    woutT = nc.dram_tensor("woutT", [HC, D], bf16, kind="ExternalInput")
    bout = nc.dram_tensor("bout", [1, D], bf16, kind="ExternalInput")
    out_d = nc.dram_tensor("out", [SHARD, D], f32, kind="ExternalOutput")

    DTMAX = max(Dts)
    chunks = []  # per tile: list of (abs_col, local_off, S)
    for t in range(NTILES):
        cs, off = [], 0
        for s in _pow2_chunks(Dts[t]):
            cs.append((int(coloff[t]) + off, off, s))
            off += s
        chunks.append(cs)

    rg = [list(range(NCORES))]

    with tile.TileContext(nc) as tc:
        with (
            tc.tile_pool(name="dram", bufs=1, space="DRAM") as dram,
            tc.tile_pool(name="const", bufs=1) as const,
        ):
            # persistent DRAM scratch
            qsd = [dram.tile([SHARD, 2 * HC], bf16, name=f"qs{l}d") for l in range(2)]
            kvin = [dram.tile([SHARD, 2 * HC], bf16, name=f"kv{l}in") for l in range(2)]
            kvfull = [dram.tile([NPAD, 2 * HC], bf16, name=f"kv{l}full", addr_space="Shared")
                      for l in range(2)]
            hT0 = dram.tile([SHARD, HC], bf16, name="h0T")

            # constants + all weights in SBUF up front
            ident = const.tile([P, P], bf16)
            make_identity(nc, ident[:])
            ones = const.tile([1, P], bf16)
            nc.vector.memset(ones[:], 1.0)
            srcidx_s = const.tile([P, SUMD], mybir.dt.int32)
            nc.sync.dma_start(srcidx_s[:], srcidx_d[:])
            ninv_s = const.tile([P, NTILES], f32)
            nc.sync.dma_start(ninv_s[:], ninv_d[:])
            w_s, bias_s = {}, {}
            for l, Din in ((0, D), (1, HC)):
                KB = Din // P
                for nm in "qkvs":
                    w_s[nm, l] = const.tile([P, KB * HC], bf16, name=f"w{nm}{l}s")
                    nc.sync.dma_start(
                        w_s[nm, l][:].rearrange("p (kb n) -> p kb n", n=HC),
                        wT[nm, l][:].rearrange("(kb p) n -> p kb n", p=P))
                    bias_s[nm, l] = const.tile([1, HC], bf16, name=f"b{nm}{l}s")
                    nc.sync.dma_start(bias_s[nm, l][:], bia[nm, l][:])
            wo_s = const.tile([P, (HC // P) * D], bf16)
            nc.sync.dma_start(
                wo_s[:].rearrange("p (kb n) -> p kb n", n=D),
                woutT[:].rearrange("(kb p) n -> p kb n", p=P))
            bout_s = const.tile([1, D], bf16)
            nc.sync.dma_start(bout_s[:], bout[:])

            def ag(l):
                nc.gpsimd.collective_compute(
                    "AllGather", mybir.AluOpType.bypass, replica_groups=rg,
                    ins=[kvin[l][:].opt()], outs=[kvfull[l][:].opt()])

            def proj_tile(l, t, group, lhsT, lp, op, pp):
                """Project one 128-row tile for group ('kv'|'qs') from a
                [P, KB*P] stationary operand already in SBUF (or load it)."""
                KB = (D if l == 0 else HC) // P
                rows = slice(t * P, (t + 1) * P)
                o = op.tile([P, 2 * HC], bf16, tag=f"o{group}")
                for j, nm in enumerate(group):
                    ps = pp.tile([P, HC], f32, tag="ps")
                    for kb in range(KB):
                        nc.tensor.matmul(
                            ps[:], lhsT=lhsT[:, kb * P:(kb + 1) * P],
                            rhs=w_s[nm, l][:].rearrange("p (kb n) -> p kb n", n=HC)[:, kb, :],
                            start=(kb == 0), stop=False)
                    nc.tensor.matmul(
                        ps[:], lhsT=ones[:1, :], rhs=bias_s[nm, l][:1, :],
                        start=False, stop=True)
                    nc.scalar.copy(o[:, j * HC:(j + 1) * HC], ps[:])
                if group == "kv":
                    nc.sync.dma_start(kvin[l][rows, :], o[:])
                else:
                    nc.sync.dma_start(qsd[l][rows, :], o[:])

            def load_lhs0(t, lp):
                KB = D // P
                lt = lp.tile([P, KB * P], bf16, tag="lhs")
                nc.sync.dma_start(lt[:], xT[t * P:(t + 1) * P, :])
                return lt

            def zpad_write(l, op):
                zp = op.tile([1, 2 * HC], bf16, tag="zpad")
                nc.vector.memset(zp[:], 0.0)
                nc.sync.dma_start(kvin[l][SHARD - 1:SHARD, :], zp[:])

            # ---------------- layer-0 projections: kv -> AG0 -> qs --------
            with (
                tc.tile_pool(name="lp0", bufs=3) as lp0,
                tc.tile_pool(name="op0", bufs=2) as op0,
                tc.tile_pool(name="pp0", bufs=2, space="PSUM") as pp0,
            ):
                for t in range(NTILES):
                    proj_tile(0, t, "kv", load_lhs0(t, lp0)[:], lp0, op0, pp0)
                zpad_write(0, op0)
                ag(0)
                for t in range(NTILES):
                    proj_tile(0, t, "qs", load_lhs0(t, lp0)[:], lp0, op0, pp0)

            # ---------------- edge phases ---------------------------------
            def edge_phase(l):
                with (
                    tc.tile_pool(name=f"ek{l}", bufs=4) as ek,
                    tc.tile_pool(name=f"eg{l}", bufs=2) as eg,
                    tc.tile_pool(name=f"eh{l}", bufs=2) as eh,
                    tc.tile_pool(name=f"eo{l}", bufs=2) as eo,
                    tc.tile_pool(name=f"et{l}", bufs=4, space="PSUM") as et,
                    tc.tile_pool(name=f"ep{l}", bufs=2, space="PSUM") as ep,
                ):
                    order = []
                    lo_i, hi_i = 0, NTILES - 1
                    while lo_i <= hi_i:
                        order.append(lo_i); lo_i += 1
                        if lo_i <= hi_i:
                            order.append(hi_i); hi_i -= 1
                    for t in order:
                        rows = slice(t * P, (t + 1) * P)
                        qs_t = eg.tile([P, 2 * HC], bf16, tag="qs")
                        nc.sync.dma_start(qs_t[:], qsd[l][rows, :])
                        ebuf = eh.tile([P, DTMAX * H], bf16, tag="ebuf")
                        msg = eh.tile([P, HC], f32, tag="msg")
                        for ci, (co, lo, S) in enumerate(chunks[t]):
                            kvg = ek.tile([P, SCHUNK * 2 * HC], bf16, tag="kvg")
                            for s in range(S):
                                nc.gpsimd.indirect_dma_start(
                                    out=kvg[:, s * 2 * HC:(s + 1) * 2 * HC],
                                    out_offset=None,
                                    in_=kvfull[l][:],
                                    in_offset=bass.IndirectOffsetOnAxis(
                                        ap=srcidx_s[:, co + s:co + s + 1], axis=0))
                            kv3 = kvg[:].rearrange("p (s kv) -> p s kv", kv=2 * HC)
                            prod = eg.tile([P, SCHUNK * HC], bf16, tag="prod")
                            nc.vector.tensor_tensor(
                                out=prod[:].rearrange("p (s n) -> p s n", n=HC)[:, :S],
                                in0=kv3[:, :S, 0:HC],
                                in1=qs_t[:, None, 0:HC].to_broadcast([P, S, HC]),
                                op=mybir.AluOpType.mult)
                            p5 = prod[:].rearrange(
                                "p (s h two c) -> p s h two c", h=H, two=2, c=C // 2)
                            foldc = eg.tile([P, SCHUNK * H * (C // 2)], bf16, tag="foldc")
                            nc.vector.tensor_tensor(
                                out=foldc[:].rearrange(
                                    "p (s h c) -> p s h c", h=H, c=C // 2)[:, :S],
                                in0=p5[:, :S, :, 0, :], in1=p5[:, :S, :, 1, :],
                                op=mybir.AluOpType.add)
                            alpha = eh.tile([P, SCHUNK * H], f32, tag="alpha")
                            nc.vector.tensor_reduce(
                                alpha[:, :S * H],
                                foldc[:, :S * H * (C // 2)].rearrange(
                                    "p (sh c) -> p sh c", c=C // 2),
                                axis=mybir.AxisListType.X, op=mybir.AluOpType.add)
                            nc.scalar.activation(
                                ebuf[:, lo * H:(lo + S) * H], alpha[:, :S * H],
                                mybir.ActivationFunctionType.Exp, scale=float(INV_SQRT_C))
                            ev = eg.tile([P, SCHUNK * HC], bf16, tag="prod")
                            nc.vector.tensor_tensor(
                                out=ev[:].rearrange("p (s h c) -> p s h c", h=H, c=C)[:, :S],
                                in0=kv3[:, :S, HC:2 * HC].rearrange("p s (h c) -> p s h c", c=C),
                                in1=ebuf[:, lo * H:(lo + S) * H]
                                    .rearrange("p (s h) -> p s h", h=H)
                                    [:, :, :, None].to_broadcast([P, S, H, C]),
                                op=mybir.AluOpType.mult)
                            n = S
                            while n > 1:
                                half = n // 2
                                nc.vector.tensor_tensor(
                                    out=ev[:, :half * HC].rearrange("p (s n) -> p s n", n=HC),
                                    in0=ev[:, :half * HC].rearrange("p (s n) -> p s n", n=HC),
                                    in1=ev[:, half * HC:n * HC].rearrange("p (s n) -> p s n", n=HC),
                                    op=mybir.AluOpType.add)
                                n = half
                            if ci == 0:
                                nc.scalar.copy(msg[:], ev[:, :HC])
                            else:
                                nc.vector.tensor_add(msg[:], msg[:], ev[:, :HC])
                        den = eh.tile([P, H], f32, tag="den")
                        nc.vector.tensor_reduce(
                            den[:], ebuf[:, :Dts[t] * H].rearrange("p (s h) -> p h s", h=H),
                            axis=mybir.AxisListType.X, op=mybir.AluOpType.add)
                        # add -#invalid (each contributed exp(0)=1); clamp off 0
                        nc.vector.tensor_add(
                            den[:], den[:], ninv_s[:, t:t + 1].to_broadcast([P, H]))
                        nc.vector.tensor_scalar_max(den[:], den[:], 1e-16)
                        rden = eh.tile([P, H], f32, tag="rden")
                        nc.vector.reciprocal(rden[:], den[:])
                        h_t = eh.tile([P, HC], bf16, tag="h")
                        nc.vector.tensor_tensor(
                            out=h_t[:].rearrange("p (h c) -> p h c", c=C),
                            in0=msg[:].rearrange("p (h c) -> p h c", c=C),
                            in1=rden[:, :, None].to_broadcast([P, H, C]),
                            op=mybir.AluOpType.mult)
                        nc.vector.tensor_add(h_t[:], h_t[:], qs_t[:, HC:2 * HC])
                        nc.scalar.activation(h_t[:], h_t[:], mybir.ActivationFunctionType.Relu)
                        o4 = eh.tile([P, (HC // P) * P], bf16, tag="tpo")
                        for kb in range(HC // P):
                            tp = et.tile([P, P], bf16, tag="tp")
                            nc.tensor.transpose(tp[:], h_t[:, kb * P:(kb + 1) * P], ident[:])
                            nc.scalar.copy(o4[:, kb * P:(kb + 1) * P], tp[:])
                        if l == 0:
                            # layer-1 k|v projection straight from the
                            # transpose; store hT0 for the later q|s pass
                            proj_tile(1, t, "kv", o4[:], None, eo, ep)
                            nc.sync.dma_start(
                                hT0[t * P:(t + 1) * P, :], o4[:])
                        else:
                            # final projection: out = h @ woutT + bout
                            ps = ep.tile([P, D], f32, tag="po")
                            for kb in range(HC // P):
                                nc.tensor.matmul(
                                    ps[:], lhsT=o4[:, kb * P:(kb + 1) * P],
                                    rhs=wo_s[:].rearrange("p (kb n) -> p kb n", n=D)[:, kb, :],
                                    start=(kb == 0), stop=False)
                            nc.tensor.matmul(ps[:], lhsT=ones[:1, :], rhs=bout_s[:1, :],
                                             start=False, stop=True)
                            oo = eo.tile([P, D], f32, tag="oo")
                            nc.scalar.copy(oo[:], ps[:])
                            nc.sync.dma_start(out_d[rows, :], oo[:])
                    if l == 0:
                        zpad_write(1, eo)

            edge_phase(0)
            ag(1)
            # q|s-1 projections from stored hT0, overlapping AG1
            with (
                tc.tile_pool(name="lp1", bufs=3) as lp1,
                tc.tile_pool(name="op1", bufs=2) as op1,
                tc.tile_pool(name="pp1", bufs=2, space="PSUM") as pp1,
            ):
                KB = HC // P
                for t in range(NTILES):
                    lt = lp1.tile([P, KB * P], bf16, tag="lhs")
                    nc.sync.dma_start(lt[:], hT0[t * P:(t + 1) * P, :])
                    proj_tile(1, t, "qs", lt[:], lp1, op1, pp1)
            edge_phase(1)

    _split_waits(nc)
    return nc


def _make_in_maps(inputs, srcidx, ninv, node_of_rank):
    x = np.ascontiguousarray(np.asarray(inputs["x"], np.float32))
    xsh = _shard_rows(x, node_of_rank)
    g = lambda n: np.ascontiguousarray(np.asarray(inputs[n], np.float32))
    common = {}
    for l in range(2):
        for nm in "qkvs":
            common[f"w{nm}{l}T"] = np.ascontiguousarray(g(f"{nm}{l}_w").T).astype(BF16)
            common[f"b{nm}{l}"] = g(f"{nm}{l}_b").reshape(1, HC).astype(BF16)
    common["woutT"] = np.ascontiguousarray(g("out_w").T).astype(BF16)
    common["bout"] = g("out_b").reshape(1, D).astype(BF16)
    in_maps = []
    for c in range(NCORES):
        m = dict(common)
        xc = xsh[c].reshape(NTILES, P, D)          # [t, node r, feat]
        xt = np.ascontiguousarray(xc.transpose(0, 2, 1))  # [t, feat, node]
        m["xT"] = xt.reshape(NTILES, D // P, P, P).transpose(0, 2, 1, 3) \
            .reshape(SHARD, D).astype(BF16)
        m["srcidx"] = np.ascontiguousarray(srcidx[c])
        m["ninv"] = np.ascontiguousarray(ninv[c])
        in_maps.append(m)
    return in_maps


def kernel(**inputs):
    from concourse.bass_utils import run_bass_kernel_spmd
    srcidx, ninv, Dts, SUMD, coloff, node_of_rank = _prep(np.asarray(inputs["edge_index"]))
    nc = _build_nc(Dts, SUMD, coloff)
    in_maps = _make_in_maps(inputs, srcidx, ninv, node_of_rank)
    res = run_bass_kernel_spmd(nc, in_maps, core_ids=list(range(NCORES)))
    shards = np.stack([res.results[c]["out"] for c in range(NCORES)])
    full = np.empty((N, D), np.float32)
    r = np.arange(N)
    full[node_of_rank] = shards[r % NCORES, r // NCORES]
    return full
